# revision 44
# baseline (speedup 1.0000x reference)
"""GAE (generalized advantage estimation) kernel for trn2, 8 NeuronCores.

Computes advantages[t] = delta[t] + gl * advantages[t+1] (reverse scan over
T-1=1023 steps) for deltas = rewards[:-1] + gamma*values[1:] - values[:-1],
for 32768 independent batch columns, data-parallel over 8 cores.

Design (bf16 I/O, 127-row time blocks, ~40.4us/core vs 129.9us baseline):
  Per core R, V are [1024, 4096] bf16 -> A [1023, 4096] bf16 (host upcasts
  to f32; inputs are rounded to bf16 on the host - rel err ~4.7e-3 vs the
  2e-2 tolerance). Time axis: 8 blocks of 127 output rows + a 7-row tail
  block. With 127-row blocks every delta in a block needs only V rows from
  the same 128-row input window, so the cross-block carry is exactly the
  next block's output row 0 (the advantage at the block boundary), with no
  gamma/gl correction term:
    out_m[i] = sum_{j=i}^{126} gl^(j-i) delta[127m+j]
             + gl^(127-i) * adv[127(m+1)]
  R tile [128, 4096]: partition 0 = carry slot (engine writes must start at
  partition 0, not 127), poked with the next block's stage row 0;
  partitions 1..127 = R rows 127m..127m+126. V tile = V rows 127m..127m+127.
  Per 1024-wide psum tile (2 banks): psum = W^T V (start) + L1c^T R (stop);
  L1c row 0 carries the gl^(127-i) coefficients. All mmV are issued before
  the mmR so only mmR waits on the carry poke. Stage copies (psum f32 ->
  bf16 SBUF, mandatory: DMA cannot read PSUM) are split DVE/Act; carry
  pokes are bf16 SBUF row copies on DVE (4x perf mode).
  DMA transfers occupy the issuing engine in the cost model, so the three
  DMA-capable queues are balanced: R loads on SP, V loads on Pool (SWDGE),
  stores mostly on Act early (while SP/Pool stream loads) and on SP/Pool
  late. A few warmup matmuls hold the PE p-state at full clock from t=0.
"""
import numpy as np

GAMMA = 0.99
LAM = 0.95
GL = GAMMA * LAM
T = 1024
B = 32768
NCORES = 8
BC = B // NCORES          # 4096 batch cols per core
P = 128                   # partitions
BP = 127                  # output rows per full time block
NB = 8                    # full blocks; block index 8 is the 7-row tail
NW = 512                  # matmul moving width (1 PSUM bank, fp32)
PW = 1024                 # psum/stage-copy granularity (2 banks)
NPT = BC // PW            # 4 psum tiles per block
SW = 2048                 # store width (2 stores per block)
NWARM = 3                 # PE p-state warmup matmuls

# Store-queue assignment for the 16 full-block stores (blocks 8..1, two
# each) and the DVE/Act stage split, tuned against the cost model.
import os as _o
_SQL = {
 "0": ["act", "act", "act", "act", "act", "act", "act", "pool", "sp",
       "pool", "sp", "pool", "sp", "pool"],
 "1": ["act", "act", "act", "act", "act", "sp", "pool", "pool", "sp",
       "pool", "sp", "pool", "sp", "pool"],
 "2": ["act", "act", "act", "act", "act", "act", "pool", "sp", "pool",
       "sp", "pool", "sp", "pool", "sp"],
 "3": ["act", "act", "act", "act", "act", "act", "act", "act", "pool",
       "sp", "pool", "sp", "pool", "sp"],
}
STORE_Q = _SQL[_o.environ.get("SQ", "0")]
_SDP = {"03": (0, 3), "02": (0, 2), "01": (0, 1), "23": (2, 3), "13": (1, 3)}
STAGE_DVE_PTIS = _SDP[__import__("os").environ.get("SD", "03")]


def _make_consts():
    # L1c [128, 128] lhsT for R tiles: row 0 = carry coeffs, row 1+j = R data
    L1c = np.zeros((P, P), np.float64)
    for i in range(BP):
        L1c[0, i] = GL ** (BP - i)
    for j in range(BP):
        L1c[1 + j, : j + 1] = GL ** (j - np.arange(j + 1))
    # W [128, 128] lhsT for V tiles
    W = np.zeros((P, P), np.float64)
    for i in range(BP):
        W[i, i] = -1.0
        if i + 1 < BP:
            W[i + 1:BP, i] = GAMMA * (1 - LAM) * GL ** (np.arange(i + 1, BP) - 1 - i)
        W[BP, i] = GAMMA * GL ** (BP - 1 - i)
    # compact tail: the [8, 4096] tail regions are loaded as [128, 256]
    # tiles (partition k = (t, s) = (k // 16, k % 16), i.e. time row
    # 1016+t, batch sub-chunk s); CTR/CTV are the block-diagonal lhsTs
    # that evaluate the 7-step tail recurrence in that layout.
    CTR = np.zeros((P, P), np.float64)
    CTV = np.zeros((P, P), np.float64)
    for i in range(112):           # outputs: t_i = i // 16 in 0..6
        ti, si = i // 16, i % 16
        for k in range(P):
            tk, sk = k // 16, k % 16
            if sk != si:
                continue
            if ti <= tk <= 6:
                CTR[k, i] = GL ** (tk - ti)
            if tk == ti:
                CTV[k, i] = -1.0
            elif ti < tk <= 6:
                CTV[k, i] = GAMMA * (1 - LAM) * GL ** (tk - 1 - ti)
            elif tk == 7:
                CTV[k, i] = GAMMA * GL ** (6 - ti)
    from ml_dtypes import bfloat16
    return tuple(np.ascontiguousarray(x, dtype=np.float32).astype(bfloat16)
                 for x in (L1c, W, CTR, CTV))


def _build(reps: int = 1):
    import concourse.bacc as bacc
    import concourse.mybir as mybir
    from concourse.tile import TileContext

    f32 = mybir.dt.float32
    bf16 = mybir.dt.bfloat16
    nc = bacc.Bacc("TRN2")
    R = nc.dram_tensor("R", [T, BC], bf16, kind="ExternalInput")
    V = nc.dram_tensor("V", [T, BC], bf16, kind="ExternalInput")
    CALL = nc.dram_tensor("CALL", [P, 4 * P], bf16, kind="ExternalInput")
    A = nc.dram_tensor("A", [T - 1, BC], bf16, kind="ExternalOutput")

    store_eng = {"sp": "sync", "act": "scalar", "pool": "gpsimd"}

    with TileContext(nc) as tc:
        with (
            tc.tile_pool(name="cst", bufs=1) as cst,
            tc.tile_pool(name="rp", bufs=4) as rp,
            tc.tile_pool(name="vp", bufs=4) as vp,
            tc.tile_pool(name="stp", bufs=4) as stp,
            tc.tile_pool(name="ps", bufs=4, space="PSUM") as ps,
        ):
            call = cst.tile([P, 4 * P], bf16, tag="call")
            nc.scalar.dma_start(out=call[:, :], in_=CALL[:, :])
            ctr = call[:, 0:P]
            ctv = call[:, P:2 * P]
            l1c = call[:, 2 * P:3 * P]
            w = call[:, 3 * P:4 * P]

            # PE p-state warmup: keep the tensor engine clocked up from t=0
            warm = cst.tile([P, NW], bf16, tag="warm")
            nc.vector.memset(warm[:, :], 0.0)
            wpt = ps.tile([P, PW], f32, tag="ps")
            for _ in range(NWARM):
                nc.tensor.matmul(wpt[:, 0:NW], warm[:, 0:P], warm[:, :],
                                 start=True, stop=True)

            def one_pass():
                # All load DMAs up front in consumption order (m = 8 .. 0).
                # R loads on the SP queue, V loads on the Pool (SWDGE) queue.
                rt, vt = {}, {}
                # compact tail tiles: the contiguous [8, 4096] tail regions
                # load as [128, 256] (512B/partition -> ~200ns DMAs)
                rc = rp.tile([P, 2 * P], bf16, tag="rc")
                vc = vp.tile([P, 2 * P], bf16, tag="vc")
                nc.sync.dma_start(out=rc[:, :], in_=R[BP * NB:T, :])
                nc.gpsimd.dma_start(out=vc[:, :], in_=V[BP * NB:T, :])

                for m in range(NB - 1, -1, -1):
                    r = rp.tile([P, BC], bf16, tag="r")
                    v = vp.tile([P, BC], bf16, tag="v")
                    # partitions 1..127 = R rows 127m..127m+126; partition 0
                    # is the carry slot (written by the poke); block 7 loads
                    # in 1024-quarters so its first mmV fires early
                    nh = 4 if m == NB - 1 else 2
                    for h in [slice(q * (BC // nh), (q + 1) * (BC // nh))
                              for q in range(nh)]:
                        nc.sync.dma_start(out=r[1:P, h],
                                          in_=R[BP * m:BP * m + BP, h])
                        nc.gpsimd.dma_start(out=v[:, h],
                                            in_=V[BP * m:BP * m + P, h])
                    rt[m], vt[m] = r, v

                # Tail first: one 256-wide psum, staged compact, its
                # adv_1016 rows (partitions 0..15) DMA-scattered into block
                # 7's carry slot, and a single ~200ns store for A[1016:].
                ptc = ps.tile([P, PW], f32, tag="ps")
                nc.tensor.matmul(ptc[:, 0:2 * P], ctv, vc[:, :],
                                 start=True, stop=False)
                nc.tensor.matmul(ptc[:, 0:2 * P], ctr, rc[:, :],
                                 start=False, stop=True)
                stc = stp.tile([P, 2 * P], bf16, tag="stc")
                if __import__("os").environ.get("TSE", "act") == "act":
                    nc.scalar.copy(stc[:, :], ptc[:, 0:2 * P])
                else:
                    nc.vector.tensor_copy(stc[:, :], ptc[:, 0:2 * P])

                # Blocks m = 7 .. 0; carry = stage row 0 poked into the next
                # (lower) block's R-tile partition 0, per 1024-wide window.
                ncopy = 0
                nstore = 0
                first = True
                for m in range(NB - 1, -1, -1):
                    lr = l1c
                    lv = w
                    r, v = rt[m], vt[m]
                    if first:
                        # carry for block 7 = adv_1016 = tail stage rows
                        # (t=0, s=0..15); sbuf->sbuf DMA remaps the 16
                        # partitions into the [1, 4096] carry slot
                        nc.scalar.dma_start(out=r[0:1, 0:SW],
                                            in_=stc[0:8, :])
                        nc.scalar.dma_start(out=r[0:1, SW:BC],
                                            in_=stc[8:16, :])
                        nc.scalar.dma_start(out=A[BP * NB:T - 1, :],
                                            in_=stc[0:112, :])
                        first = False
                    st = stp.tile([P, BC], bf16, tag="st")
                    nrow = BP
                    base = BP * m
                    pts = []
                    # all mmV first: carry-independent PE work
                    for pti in range(NPT):
                        pt = ps.tile([P, PW], f32, tag="ps")
                        pts.append(pt)
                        for sc in range(2):
                            fs = slice(pti * PW + sc * NW,
                                       pti * PW + (sc + 1) * NW)
                            nc.tensor.matmul(pt[:, sc * NW:(sc + 1) * NW],
                                             lv, v[:, fs],
                                             start=True, stop=False)
                    for pti in range(NPT):
                        pt = pts[pti]
                        ws = slice(pti * PW, (pti + 1) * PW)
                        for sc in range(2):
                            fs = slice(pti * PW + sc * NW,
                                       pti * PW + (sc + 1) * NW)
                            nc.tensor.matmul(pt[:, sc * NW:(sc + 1) * NW],
                                             lr, r[:, fs],
                                             start=False, stop=True)
                        # psum->bf16 stage copy (1024 wide), split DVE/Act
                        ncopy += 1
                        if m == 0:
                            # final block: alternate engines per psum tile
                            # and store immediately to shorten the tail
                            if pti % 2 == 0:
                                nc.vector.tensor_copy(st[:, ws], pt[:, :])
                            else:
                                nc.scalar.copy(st[:, ws], pt[:, :])
                            eng = getattr(nc, store_eng[
                                ("sp", "pool", "act", "sp")[pti]])
                            eng.dma_start(out=A[0:BP, ws], in_=st[0:BP, ws])
                        elif pti in STAGE_DVE_PTIS:
                            nc.vector.tensor_copy(st[:, ws], pt[:, :])
                        else:
                            nc.scalar.copy(st[:, ws], pt[:, :])
                        if m > 0:
                            # carry poke: bf16 sbuf row copy (DVE 4x mode)
                            nc.vector.tensor_copy(rt[m - 1][0:1, ws],
                                                  st[0:1, ws])
                    if m != 0:
                        for si in range(BC // SW):
                            cs = slice(si * SW, (si + 1) * SW)
                            eng = getattr(
                                nc, store_eng[STORE_Q[nstore % len(STORE_Q)]])
                            nstore += 1
                            eng.dma_start(out=A[base:base + nrow, cs],
                                          in_=st[0:nrow, cs])

            for _ in range(reps):
                one_pass()
    nc.finalize()
    return nc


_NC_CACHE = None


def kernel(rewards: np.ndarray, values: np.ndarray) -> np.ndarray:
    from ml_dtypes import bfloat16
    from concourse.bass_utils import run_bass_kernel_spmd

    rewards = np.asarray(rewards)
    values = np.asarray(values)

    global _NC_CACHE
    if _NC_CACHE is None:
        _NC_CACHE = _build()
    nc = _NC_CACHE

    L1c, W, CTR, CTV = _make_consts()
    call = np.ascontiguousarray(np.concatenate([CTR, CTV, L1c, W], axis=1))
    in_maps = []
    for c in range(NCORES):
        cs = slice(c * BC, (c + 1) * BC)
        in_maps.append({
            "R": np.ascontiguousarray(rewards[:, cs], dtype=np.float32).astype(bfloat16),
            "V": np.ascontiguousarray(values[:, cs], dtype=np.float32).astype(bfloat16),
            "CALL": call,
        })
    res = run_bass_kernel_spmd(nc, in_maps, core_ids=list(range(NCORES)))
    out = np.empty((T - 1, B), dtype=np.float32)
    for c in range(NCORES):
        out[:, c * BC:(c + 1) * BC] = res.results[c]["A"].astype(np.float32)
    return out
